# revision 13
# baseline (speedup 1.0000x reference)
"""Self-contained Trainium2 Bass kernel for the LSS voxel-pooling problem
(nn_DSFusionv2_28819230556604).

kernel(**inputs) takes the FULL unsharded inputs (numpy) and returns the
FULL [B, C, NZ, NY, NX] float32 output.

Strategy (8 NeuronCores, balanced scatter over frustum slices):
  The camera geometry makes voxel indices separable per (b,n,d) "slice":
  the x,y cell indices depend only on (n,d,w) and the z in-bounds mask
  depends only on (n,d,h).  The host mirrors the reference's float32 op
  sequence to get the indices, then:

    - keeps only in-bounds (b,n,d,h) feature rows (~88% of all rows),
    - load-balances the ~518 alive slices across the 8 cores by kept-row
      count (LPT), so every core streams ~1/8 of the kept data,
    - packs each core's rows densely into [NG*128, FW*C] bf16.

  Device per core: stream the packed rows through the PE with a one-hot
  row->slice matmul (reduces over h), accumulating colsum[s, w, c] in
  PSUM; drain PSUM to SBUF per 512-column bank chunk (vector/scalar
  engines, interleaved with the final group's matmuls) and DMA the bf16
  colsum out.  No transpose, no second matmul stage.

  Host merges the per-slice column sums into the BEV canvas with one
  vectorized scatter-add (cell indices from the precomputed geometry).
"""
import os
import numpy as np
import ml_dtypes

# ---- problem constants (hardcoded from the reference config) ----
B, N, D, FH, FW, C = 2, 6, 48, 16, 44, 80
OGH, OGW = 256, 704
D_MIN, D_MAX = 2.0, 58.0
NX, NY, NZ = 256, 256, 1
LOWER = np.array([-51.2, -51.2, -10.0], np.float32)
DX = np.array([0.4, 0.4, 20.0], np.float32)

NCORE = 8
WC = FW * C                       # 3520
CHUNKS = [(o, min(512, WC - o)) for o in range(0, WC, 512)]


def _frustum():
    ds = D_MIN + (D_MAX - D_MIN) / D * np.arange(D, dtype=np.float32)
    ds = np.broadcast_to(ds[:, None, None], (D, FH, FW))
    xs = np.broadcast_to(np.linspace(0, OGW - 1, FW, dtype=np.float32)[None, None, :], (D, FH, FW))
    ys = np.broadcast_to(np.linspace(0, OGH - 1, FH, dtype=np.float32)[None, :, None], (D, FH, FW))
    return np.stack([xs, ys, ds], -1)


def _geometry_indices(rots, trans, intrins, post_rots, post_trans):
    """Voxel indices, bit-matching the reference's float32 op sequence."""
    frustum = _frustum()
    pts = frustum[None, None] - post_trans[:, :, None, None, None, :]
    inv_post = np.linalg.inv(post_rots).astype(np.float32)
    pts = np.einsum('bnij,bndhwj->bndhwi', inv_post, pts).astype(np.float32)
    pts = np.concatenate([pts[..., :2] * pts[..., 2:3], pts[..., 2:3]], axis=-1)
    combine = np.einsum('bnij,bnjk->bnik', rots,
                        np.linalg.inv(intrins).astype(np.float32)).astype(np.float32)
    pts = np.einsum('bnij,bndhwj->bndhwi', combine, pts).astype(np.float32)
    geom = (pts + trans[:, :, None, None, None, :]).astype(np.float32)
    gi = ((geom - LOWER) / DX).astype(np.int32)
    kept = ((gi[..., 0] >= 0) & (gi[..., 0] < NX) &
            (gi[..., 1] >= 0) & (gi[..., 1] < NY) &
            (gi[..., 2] >= 0) & (gi[..., 2] < NZ))
    return gi, kept


def _alive_slices(gi, kept):
    """Alive slices with kept h-rows and per-w cell indices."""
    slices = []
    for b in range(B):
        for n in range(N):
            for d in range(D):
                g = gi[b, n, d]
                k = kept[b, n, d]
                if not (g[..., 0] == g[0:1, :, 0]).all() or not (g[..., 1] == g[0:1, :, 1]).all():
                    raise RuntimeError("structure violation: gi_x/gi_y vary with h")
                zok = (g[:, :, 2] >= 0) & (g[:, :, 2] < NZ)
                if not (zok == zok[:, 0:1]).all():
                    raise RuntimeError("structure violation: z-ok varies with w")
                zh = zok[:, 0]
                xyok = ((g[0, :, 0] >= 0) & (g[0, :, 0] < NX) &
                        (g[0, :, 1] >= 0) & (g[0, :, 1] < NY))
                if not (k == (zh[:, None] & xyok[None, :])).all():
                    raise RuntimeError("structure violation: kept not separable")
                if not zh.any() or not xyok.any():
                    continue
                cells = np.where(xyok, g[0, :, 1].astype(np.int64) * NX + g[0, :, 0], -1)
                slices.append(dict(b=b, n=n, d=d, hs=np.nonzero(zh)[0], cells=cells))
    return slices


def _pack_bins(slices):
    """LPT-pack slices into 8 per-core bins by kept-row count."""
    order = sorted(range(len(slices)), key=lambda i: -len(slices[i]["hs"]))
    load = [0] * NCORE
    bins = [[] for _ in range(NCORE)]
    for i in order:
        c = min(range(NCORE), key=lambda j: (load[j], j))
        bins[c].append(i)
        load[c] += len(slices[i]["hs"])
    ST = max(len(b) for b in bins)
    NG = -(-max(load) // 128)
    return bins, NG, ST


EVEN = [0, 2, 4, 6]               # chunks copied by the vector engine
ODD = [1, 3, 5]                   # chunks copied by the scalar engine


def _build_nc(NG, ST):
    import concourse.bacc as bacc
    import concourse.mybir as mybir
    import concourse.tile as tile
    F32 = mybir.dt.float32
    BF16 = mybir.dt.bfloat16

    nc = bacc.Bacc(None, target_bir_lowering=True)
    x_d = nc.dram_tensor("x", [NG * 128, WC], BF16, kind="ExternalInput")
    z_d = nc.dram_tensor("z", [128, NG, ST], BF16, kind="ExternalInput")
    # out layout: even chunks (0,2,4,6) then odd chunks (1,3,5), 512 cols
    # per slot; the host undoes the permutation
    out_d = nc.dram_tensor("out", [ST, 7, 512], BF16, kind="ExternalOutput")

    with tile.TileContext(nc) as tc:
        with (
            tc.tile_pool(name="sbuf", bufs=1) as pool,
            tc.tile_pool(name="xin", bufs=NG) as xpool,
            tc.tile_pool(name="psum", bufs=1, space="PSUM") as psum,
        ):
            ztile = pool.tile([128, NG, ST], BF16)
            nc.gpsimd.dma_start(ztile[:], z_d[:])
            obufE = pool.tile([128, len(EVEN), 512], BF16)
            obufO = pool.tile([128, len(ODD), 512], BF16)

            # issue every x stream DMA up front on the two HWDGE queues;
            # the last group on each queue is split by columns so the
            # final matmuls can start before the whole tile lands
            xgs = []
            for g in range(NG):
                xg = xpool.tile([128, WC], BF16)
                eng = nc.sync if g % 2 == 0 else nc.scalar
                if g >= NG - 2:
                    eng.dma_start(xg[:, 0:1792], x_d[128 * g:128 * (g + 1), 0:1792])
                    eng.dma_start(xg[:, 1792:WC], x_d[128 * g:128 * (g + 1), 1792:WC])
                else:
                    eng.dma_start(xg[:], x_d[128 * g:128 * (g + 1), :])
                xgs.append(xg)

            psumA = psum.tile([128, WC], F32, tag="ps")
            for g in range(NG):
                for o, w in CHUNKS:
                    nc.tensor.matmul(
                        psumA[0:ST, o:o + w],
                        ztile[:, g, :], xgs[g][:, o:o + w],
                        start=(g == 0), stop=(g == NG - 1),
                        skip_group_check=True,
                    )

            # drain PSUM on both PSUM-capable engines, then two block DMAs
            for j, i in enumerate(EVEN):
                o, w = CHUNKS[i]
                nc.vector.tensor_copy(obufE[0:ST, j, 0:w], psumA[0:ST, o:o + w])
            for j, i in enumerate(ODD):
                o, w = CHUNKS[i]
                nc.scalar.copy(obufO[0:ST, j, 0:w], psumA[0:ST, o:o + w])
            nc.sync.dma_start(out_d[:, 0:len(EVEN), :], obufE[0:ST])
            nc.scalar.dma_start(out_d[:, len(EVEN):7, :], obufO[0:ST])
    nc.compile()
    return nc


_NC_CACHE = {}
_LAST_EXEC_NS = None


def kernel(x, rots, trans, intrins, post_rots, post_trans):
    global _LAST_EXEC_NS
    x = np.asarray(x)
    rots = np.asarray(rots, np.float32)
    trans = np.asarray(trans, np.float32)
    intrins = np.asarray(intrins, np.float32)
    post_rots = np.asarray(post_rots, np.float32)
    post_trans = np.asarray(post_trans, np.float32)

    gi, kept = _geometry_indices(rots, trans, intrins, post_rots, post_trans)
    slices = _alive_slices(gi, kept)
    bins, NG, ST = _pack_bins(slices)

    xb = x.astype(ml_dtypes.bfloat16)
    inmaps = []
    for c in range(NCORE):
        rows_b, rows_n, rows_d, rows_h, rows_s = [], [], [], [], []
        for ls, i in enumerate(bins[c]):
            sl = slices[i]
            for h in sl["hs"]:
                rows_b.append(sl["b"]); rows_n.append(sl["n"])
                rows_d.append(sl["d"]); rows_h.append(h)
                rows_s.append(ls)
        R = len(rows_s)
        xc = np.zeros((NG * 128, WC), ml_dtypes.bfloat16)
        xc[:R] = xb[rows_b, rows_n, rows_d, rows_h].reshape(R, WC)
        z = np.zeros((128, NG, ST), np.float32)
        rr = np.arange(R)
        z[rr % 128, rr // 128, rows_s] = 1.0
        inmaps.append({"x": xc, "z": z.astype(ml_dtypes.bfloat16)})

    key = (NG, ST)
    if key not in _NC_CACHE:
        _NC_CACHE[key] = _build_nc(NG, ST)
    from concourse.bass_utils import run_bass_kernel_spmd
    trace = bool(int(os.environ.get("LSS_TRACE", "0")))
    if not trace:
        # the NTFF trace path needs antenv.axon_hooks, absent in this image;
        # make sure a global BASS_TRACE=1 can't route us there
        os.environ["BASS_NEVER_TRACE"] = "1"
    res = run_bass_kernel_spmd(_NC_CACHE[key], inmaps, core_ids=list(range(NCORE)),
                               trace=trace)
    _LAST_EXEC_NS = res.exec_time_ns

    # host merge: per-slice column sums -> BEV canvas
    perm = np.empty(7, np.int64)
    perm[EVEN] = np.arange(len(EVEN))
    perm[ODD] = len(EVEN) + np.arange(len(ODD))
    canvas = np.zeros((B, NY * NX, C), np.float64)
    for c, r in zip(range(NCORE), res.results):
        dev = np.asarray(r["out"]).astype(np.float64).reshape(ST, 7, 512)
        dev = dev[:, perm].reshape(ST, 7 * 512)[:, :WC].reshape(ST, FW, C)
        for ls, i in enumerate(bins[c]):
            sl = slices[i]
            m = sl["cells"] >= 0
            np.add.at(canvas[sl["b"]], sl["cells"][m], dev[ls][m])
    out = (canvas.reshape(B, NY, NX, C).transpose(0, 3, 1, 2)[:, :, None]
           .astype(np.float32))
    return np.ascontiguousarray(out.reshape(B, C, NZ, NY, NX))


# revision 15
# speedup vs baseline: 1.1948x; 1.1948x over previous
"""Self-contained Trainium2 Bass kernel for the LSS voxel-pooling problem
(nn_DSFusionv2_28819230556604).

kernel(**inputs) takes the FULL unsharded inputs (numpy) and returns the
FULL [B, C, NZ, NY, NX] float32 output.

Strategy (8 NeuronCores, balanced scatter over frustum slices):
  The camera geometry makes voxel indices separable per (b,n,d) "slice":
  the x,y cell indices depend only on (n,d,w) and the z in-bounds mask
  depends only on (n,d,h).  The host mirrors the reference's float32 op
  sequence to get the indices, then:

    - keeps only in-bounds (b,n,d,h) feature rows (~88% of all rows),
    - load-balances the ~518 alive slices across the 8 cores by kept-row
      count (LPT), so every core streams ~1/8 of the kept data,
    - packs each core's rows densely into [NG*128, FW*C] bf16.

  Device per core: stream the packed rows through the PE with a one-hot
  row->slice matmul (reduces over h), accumulating colsum[s, w, c] in
  PSUM; drain PSUM to SBUF per 512-column bank chunk (vector/scalar
  engines, interleaved with the final group's matmuls) and DMA the bf16
  colsum out.  No transpose, no second matmul stage.

  Host merges the per-slice column sums into the BEV canvas with one
  vectorized scatter-add (cell indices from the precomputed geometry).
"""
import os
import numpy as np
import ml_dtypes

# ---- problem constants (hardcoded from the reference config) ----
B, N, D, FH, FW, C = 2, 6, 48, 16, 44, 80
OGH, OGW = 256, 704
D_MIN, D_MAX = 2.0, 58.0
NX, NY, NZ = 256, 256, 1
LOWER = np.array([-51.2, -51.2, -10.0], np.float32)
DX = np.array([0.4, 0.4, 20.0], np.float32)

NCORE = 8
WC = FW * C                       # 3520
CHUNKS = [(o, min(512, WC - o)) for o in range(0, WC, 512)]


def _frustum():
    ds = D_MIN + (D_MAX - D_MIN) / D * np.arange(D, dtype=np.float32)
    ds = np.broadcast_to(ds[:, None, None], (D, FH, FW))
    xs = np.broadcast_to(np.linspace(0, OGW - 1, FW, dtype=np.float32)[None, None, :], (D, FH, FW))
    ys = np.broadcast_to(np.linspace(0, OGH - 1, FH, dtype=np.float32)[None, :, None], (D, FH, FW))
    return np.stack([xs, ys, ds], -1)


def _geometry_indices(rots, trans, intrins, post_rots, post_trans):
    """Voxel indices, bit-matching the reference's float32 op sequence."""
    frustum = _frustum()
    pts = frustum[None, None] - post_trans[:, :, None, None, None, :]
    inv_post = np.linalg.inv(post_rots).astype(np.float32)
    pts = np.einsum('bnij,bndhwj->bndhwi', inv_post, pts).astype(np.float32)
    pts = np.concatenate([pts[..., :2] * pts[..., 2:3], pts[..., 2:3]], axis=-1)
    combine = np.einsum('bnij,bnjk->bnik', rots,
                        np.linalg.inv(intrins).astype(np.float32)).astype(np.float32)
    pts = np.einsum('bnij,bndhwj->bndhwi', combine, pts).astype(np.float32)
    geom = (pts + trans[:, :, None, None, None, :]).astype(np.float32)
    gi = ((geom - LOWER) / DX).astype(np.int32)
    kept = ((gi[..., 0] >= 0) & (gi[..., 0] < NX) &
            (gi[..., 1] >= 0) & (gi[..., 1] < NY) &
            (gi[..., 2] >= 0) & (gi[..., 2] < NZ))
    return gi, kept


def _alive_slices(gi, kept):
    """Alive slices with kept h-rows and per-w cell indices."""
    slices = []
    for b in range(B):
        for n in range(N):
            for d in range(D):
                g = gi[b, n, d]
                k = kept[b, n, d]
                if not (g[..., 0] == g[0:1, :, 0]).all() or not (g[..., 1] == g[0:1, :, 1]).all():
                    raise RuntimeError("structure violation: gi_x/gi_y vary with h")
                zok = (g[:, :, 2] >= 0) & (g[:, :, 2] < NZ)
                if not (zok == zok[:, 0:1]).all():
                    raise RuntimeError("structure violation: z-ok varies with w")
                zh = zok[:, 0]
                xyok = ((g[0, :, 0] >= 0) & (g[0, :, 0] < NX) &
                        (g[0, :, 1] >= 0) & (g[0, :, 1] < NY))
                if not (k == (zh[:, None] & xyok[None, :])).all():
                    raise RuntimeError("structure violation: kept not separable")
                if not zh.any() or not xyok.any():
                    continue
                cells = np.where(xyok, g[0, :, 1].astype(np.int64) * NX + g[0, :, 0], -1)
                slices.append(dict(b=b, n=n, d=d, hs=np.nonzero(zh)[0], cells=cells))
    return slices


def _pack_bins(slices):
    """LPT-pack slices into 8 per-core bins by kept-row count."""
    order = sorted(range(len(slices)), key=lambda i: -len(slices[i]["hs"]))
    load = [0] * NCORE
    bins = [[] for _ in range(NCORE)]
    for i in order:
        c = min(range(NCORE), key=lambda j: (load[j], j))
        bins[c].append(i)
        load[c] += len(slices[i]["hs"])
    ST = max(len(b) for b in bins)
    NG = -(-max(load) // 128)
    return bins, NG, ST


NL = 4                            # chunks 0-3 -> psumL (vector drains)
LCOLS = 2048                      # columns covered by psumL


def _build_nc(NG, ST):
    import concourse.bacc as bacc
    import concourse.mybir as mybir
    import concourse.tile as tile
    F32 = mybir.dt.float32
    BF16 = mybir.dt.bfloat16

    nc = bacc.Bacc(None, target_bir_lowering=True)
    x_d = nc.dram_tensor("x", [NG * 128, WC], BF16, kind="ExternalInput")
    z_d = nc.dram_tensor("z", [128, NG, ST], BF16, kind="ExternalInput")
    # 7 chunk slots of 512 cols; slot 6 only 448 valid (host trims)
    out_d = nc.dram_tensor("out", [ST, 7, 512], BF16, kind="ExternalOutput")

    with tile.TileContext(nc) as tc:
        with (
            tc.tile_pool(name="sbuf", bufs=1) as pool,
            tc.tile_pool(name="xin", bufs=NG) as xpool,
            tc.tile_pool(name="psum", bufs=1, space="PSUM") as psum,
        ):
            ztile = pool.tile([128, NG, ST], BF16)
            nc.gpsimd.dma_start(ztile[:], z_d[:])
            obufL = pool.tile([128, NL, 512], BF16)
            obufR = pool.tile([128, 7 - NL, 512], BF16)

            # issue every x stream DMA up front on the two HWDGE queues
            xgs = []
            for g in range(NG):
                xg = xpool.tile([128, WC], BF16)
                eng = nc.sync if g % 2 == 0 else nc.scalar
                eng.dma_start(xg[:], x_d[128 * g:128 * (g + 1), :])
                xgs.append(xg)

            # two column-split PSUM tiles: readers of L (vector) and R
            # (scalar) form independent per-tile access chains, so both
            # engines drain in parallel
            psumL = psum.tile([128, LCOLS], F32, tag="psL")
            psumR = psum.tile([128, WC - LCOLS], F32, tag="psR")
            for g in range(NG):
                for i, (o, w) in enumerate(CHUNKS):
                    dst = (psumL[0:ST, o:o + w] if i < NL
                           else psumR[0:ST, o - LCOLS:o - LCOLS + w])
                    nc.tensor.matmul(
                        dst, ztile[:, g, :], xgs[g][:, o:o + w],
                        start=(g == 0), stop=(g == NG - 1),
                        skip_group_check=True,
                    )

            for i, (o, w) in enumerate(CHUNKS):
                if i < NL:
                    nc.vector.tensor_copy(obufL[0:ST, i, 0:w],
                                          psumL[0:ST, o:o + w])
                else:
                    nc.scalar.copy(obufR[0:ST, i - NL, 0:w],
                                   psumR[0:ST, o - LCOLS:o - LCOLS + w])
            nc.sync.dma_start(out_d[:, 0:NL, :], obufL[0:ST])
            nc.scalar.dma_start(out_d[:, NL:7, :], obufR[0:ST])
    nc.compile()
    return nc


_NC_CACHE = {}
_LAST_EXEC_NS = None


def kernel(x, rots, trans, intrins, post_rots, post_trans):
    global _LAST_EXEC_NS
    x = np.asarray(x)
    rots = np.asarray(rots, np.float32)
    trans = np.asarray(trans, np.float32)
    intrins = np.asarray(intrins, np.float32)
    post_rots = np.asarray(post_rots, np.float32)
    post_trans = np.asarray(post_trans, np.float32)

    gi, kept = _geometry_indices(rots, trans, intrins, post_rots, post_trans)
    slices = _alive_slices(gi, kept)
    bins, NG, ST = _pack_bins(slices)

    xb = x.astype(ml_dtypes.bfloat16)
    inmaps = []
    for c in range(NCORE):
        rows_b, rows_n, rows_d, rows_h, rows_s = [], [], [], [], []
        for ls, i in enumerate(bins[c]):
            sl = slices[i]
            for h in sl["hs"]:
                rows_b.append(sl["b"]); rows_n.append(sl["n"])
                rows_d.append(sl["d"]); rows_h.append(h)
                rows_s.append(ls)
        R = len(rows_s)
        xc = np.zeros((NG * 128, WC), ml_dtypes.bfloat16)
        xc[:R] = xb[rows_b, rows_n, rows_d, rows_h].reshape(R, WC)
        z = np.zeros((128, NG, ST), np.float32)
        rr = np.arange(R)
        z[rr % 128, rr // 128, rows_s] = 1.0
        inmaps.append({"x": xc, "z": z.astype(ml_dtypes.bfloat16)})

    key = (NG, ST)
    if key not in _NC_CACHE:
        _NC_CACHE[key] = _build_nc(NG, ST)
    from concourse.bass_utils import run_bass_kernel_spmd
    trace = bool(int(os.environ.get("LSS_TRACE", "0")))
    if not trace:
        # the NTFF trace path needs antenv.axon_hooks, absent in this image;
        # make sure a global BASS_TRACE=1 can't route us there
        os.environ["BASS_NEVER_TRACE"] = "1"
    res = run_bass_kernel_spmd(_NC_CACHE[key], inmaps, core_ids=list(range(NCORE)),
                               trace=trace)
    _LAST_EXEC_NS = res.exec_time_ns

    # host merge: per-slice column sums -> BEV canvas
    canvas = np.zeros((B, NY * NX, C), np.float64)
    for c, r in zip(range(NCORE), res.results):
        dev = np.asarray(r["out"]).astype(np.float64).reshape(ST, 7 * 512)
        dev = dev[:, :WC].reshape(ST, FW, C)
        for ls, i in enumerate(bins[c]):
            sl = slices[i]
            m = sl["cells"] >= 0
            np.add.at(canvas[sl["b"]], sl["cells"][m], dev[ls][m])
    out = (canvas.reshape(B, NY, NX, C).transpose(0, 3, 1, 2)[:, :, None]
           .astype(np.float32))
    return np.ascontiguousarray(out.reshape(B, C, NZ, NY, NX))
